# revision 42
# baseline (speedup 1.0000x reference)
"""Causal self-attention (B=2, T=2048, C=1024, NH=16, HD=64) on 8 TRN2 cores.

Sharding: core c -> batch b = c//4, head group j = c%4 (4 heads: 4j..4j+3).
Each core computes its batch's QKV projection for its 4 heads, rope, causal
flash-style attention in S^T layout (k on partitions, q on free dim), and a
partial output projection y_part^T = Wp_slice^T.T @ attbf. Host sums the 4
per-batch partials (bf16) and adds b_proj.

v2 schedule (vs v1):
  - xT DMA chunked per (tc4, ct) and ordered by consumption so phase B
    starts ~5us in instead of ~35us; 3 DGE queues (sync/gpsimd/vector).
  - PE warmup spinner at t=0 (junk matmuls) so HAM is at K=8/8 when real
    matmuls arrive.
  - Causal trimming: diagonal S/exp/PV restricted to cols >= 128d; mask
    multiply shrunk to the single 128-col triangular block (tri mask is
    d-independent).
  - Per-qc normalize fused from PV psum (rowsum row 64 -> recip -> DRAM
    bounce broadcast -> attbf = pv * recip), pipelined into the next qc.
  - Phase E (output proj) per-qc chunks interleaved into D1's exp gaps.
  - rope bf16, masks [128,256] bf16, yT output bf16.
"""
import numpy as np
import ml_dtypes
from contextlib import ExitStack

import concourse.bass as bass
import concourse.mybir as mybir
import concourse.tile as tile
from concourse import bacc
from concourse.bass_utils import run_bass_kernel_spmd

F32 = mybir.dt.float32
BF16 = mybir.dt.bfloat16
AF = mybir.ActivationFunctionType
ALU = mybir.AluOpType

B, T, C = 2, 2048, 1024
NH, HD = 16, 64
TL = 2048          # per-core token count (one batch)
NCT = C // 128     # 8 contraction tiles
NTC = TL // 512    # 4 t-chunks of 512
NTT = TL // 128    # 16 token tiles of 128

TRACE = False      # set by test harness for profiling runs
_CACHE = {}


def _build_nc():
    nc = bacc.Bacc("TRN2", target_bir_lowering=False, debug=False)
    xT_d = nc.dram_tensor("xT", [128, NCT, TL], BF16, kind="ExternalInput").ap()
    wqk_d = nc.dram_tensor("wqkT", [128, 4, NCT, 128], BF16, kind="ExternalInput").ap()
    wv_d = nc.dram_tensor("wvT", [128, NCT, 256], BF16, kind="ExternalInput").ap()
    bqk_d = nc.dram_tensor("bqk", [128, 4], F32, kind="ExternalInput").ap()
    bv_d = nc.dram_tensor("bv", [128, 256], F32, kind="ExternalInput").ap()
    rope_d = nc.dram_tensor("rope", [128, TL], BF16, kind="ExternalInput").ap()
    masks_d = nc.dram_tensor("masks", [128, 256], BF16, kind="ExternalInput").ap()
    wp_d = nc.dram_tensor("wpT", [128, 2, 1024], BF16, kind="ExternalInput").ap()
    yT_d = nc.dram_tensor("yT", [1024, TL], BF16, kind="ExternalOutput").ap()
    rs_dram = nc.dram_tensor("rs_scratch", [4, TL], F32)

    with tile.TileContext(nc) as tc, ExitStack() as ctx:
        sb = ctx.enter_context(tc.tile_pool(name="sb", bufs=1))
        ptp = ctx.enter_context(tc.tile_pool(name="ptp", bufs=8))
        ytp = ctx.enter_context(tc.tile_pool(name="ytp", bufs=8))
        atp = ctx.enter_context(tc.tile_pool(name="atp", bufs=2))

        xT = sb.tile([128, NCT, TL], BF16)
        wqk = sb.tile([128, 4, NCT, 128], BF16)
        wv = sb.tile([128, NCT, 256], BF16)
        bqk = sb.tile([128, 4], F32)
        bv = sb.tile([128, 256], F32)
        rope = sb.tile([128, TL], BF16)
        masks = sb.tile([128, 256], BF16)
        wp = sb.tile([128, 2, 1024], BF16)
        q_sb = sb.tile([128, 2, TL], BF16)
        k_sb = sb.tile([128, 2, TL], BF16)
        v_sb = sb.tile([128, 4 * NTT, 65], BF16)
        attbf = sb.tile([128, 2, TL], BF16)
        rsb_sb = sb.tile([64, 4, TL], F32)
        junk = sb.tile([128, 512], BF16)
        ones_sb = sb.tile([128, 64], F32)

        # ---- memsets (DVE/GpSimd, no deps) ----
        nc.vector.memset(junk, 0.0)
        nc.vector.memset(ones_sb, 1.0)
        nc.vector.memset(v_sb[:, :, 64:65], 1.0)  # ones col last: rowsum row 64

        # ---- input DMAs, ordered by consumption, across 3 DGE queues ----
        qs = [nc.sync, nc.gpsimd, nc.scalar]
        # B-block-0 deps first: wqk m-slices + xT tc4=0, then rope/wv
        nc.sync.dma_start(out=wqk[:, 2], in_=wqk_d[:, 2])
        nc.gpsimd.dma_start(out=wqk[:, 0], in_=wqk_d[:, 0])
        nc.scalar.dma_start(out=bqk, in_=bqk_d)
        for ct in range(NCT):                      # tc4=0 chunk of x
            qs[ct % 3].dma_start(out=xT[:, ct, 0:512], in_=xT_d[:, ct, 0:512])
        nc.scalar.dma_start(out=rope, in_=rope_d)
        nc.sync.dma_start(out=wv, in_=wv_d)
        nc.gpsimd.dma_start(out=bv, in_=bv_d)
        nc.scalar.dma_start(out=masks, in_=masks_d)
        nc.sync.dma_start(out=wqk[:, 3], in_=wqk_d[:, 3])
        nc.gpsimd.dma_start(out=wqk[:, 1], in_=wqk_d[:, 1])
        for tc4 in range(1, NTC):                  # remaining x chunks
            for ct in range(NCT):                  # sync/gpsimd only: keep
                sl = slice(tc4 * 512, (tc4 + 1) * 512)   # scalar free for exp
                qs[(tc4 * NCT + ct) % 2].dma_start(
                    out=xT[:, ct, sl], in_=xT_d[:, ct, sl])
        nc.gpsimd.dma_start(out=wp, in_=wp_d)

        def phase_b_block(ms, tc4, pool, fi0=0):
            """QK projection for one 512-col t-chunk of the given m-tiles.
            m: 0=q hp0, 1=q hp1, 2=k hp0, 3=k hp1."""
            sl = slice(tc4 * 512, (tc4 + 1) * 512)
            for fi, m in enumerate(ms, start=fi0):
                ps = pool.tile([128, 512], F32, tag=f"f{fi % 2}",
                               name=f"pbb_{m}_{tc4}")
                for ct in range(NCT):
                    nc.tensor.matmul(
                        ps, wqk[:, m, ct, :], xT[:, ct, sl],
                        start=(ct == 0), stop=(ct == NCT - 1),
                        skip_group_check=True)
                dest = q_sb if m < 2 else k_sb
                nc.vector.scalar_tensor_tensor(
                    out=dest[:, m % 2, sl], in0=ps, scalar=bqk[:, m:m + 1],
                    in1=rope[:, sl], op0=ALU.add, op1=ALU.mult)

        def phase_b_fillers_for(pairs, pool):
            """QK projection (m,tc4) pairs sliced into 4-MM filler chunks."""
            state = {}
            chunks = []
            for m, tc4 in pairs:
                    for half in range(2):
                        def emit(m=m, tc4=tc4, half=half):
                            sl = slice(tc4 * 512, (tc4 + 1) * 512)
                            if half == 0:
                                state[(m, tc4)] = pool.tile(
                                    [128, 512], F32, tag=f"f{(m + tc4) % 2}",
                                    name=f"pbf_{m}_{tc4}")
                            ps = state[(m, tc4)]
                            cts = (0, 1, 2, 3) if half == 0 else (4, 5, 6, 7)
                            for ct in cts:
                                nc.tensor.matmul(
                                    ps, wqk[:, m, ct, :],
                                    xT[:, ct, sl],
                                    start=(ct == 0), stop=(ct == NCT - 1),
                                    skip_group_check=True)
                            if half == 1:
                                dest = q_sb if m < 2 else k_sb
                                nc.vector.scalar_tensor_tensor(
                                    out=dest[:, m % 2, sl], in0=ps,
                                    scalar=bqk[:, m:m + 1], in1=rope[:, sl],
                                    op0=ALU.add, op1=ALU.mult)
                        chunks.append(emit)
            return chunks

        def phase_c_tile(tt, pool):
            """V projection for one token tile (8 MMs + fused bias add)."""
            ps = pool.tile([128, 512], F32, tag=f"f{tt % 2}", name=f"pc_{tt}")[:, 0:256]
            for ct in range(NCT):
                nc.tensor.matmul(
                    ps, xT[:, ct, tt * 128:(tt + 1) * 128], wv[:, ct, :],
                    start=(ct == 0), stop=(ct == NCT - 1),
                    skip_group_check=True)
            nc.vector.tensor_add(
                v_sb[:, 4 * tt:4 * tt + 4, 0:64],
                ps.rearrange("p (a b) -> p a b", a=4),
                bv.rearrange("p (a b) -> p a b", a=4))

        def normalize_qc(hp, qc, pv, pool):
            """One fast pv->sbuf copy (releases the psum tile), then recip +
            broadcast + scale off the psum path. pv rows: 0-63 = att dims,
            64 = rowsum (ones col last). Broadcast of the reciprocal row:
            DRAM bounce in D0 (PE is the wall there), PE ones-matmul in D1
            (PE has slack; kills ~4us of DMA latency per chain)."""
            qsl = slice(qc * 512, (qc + 1) * 512)
            at = atp.tile([65, 2, 512], F32, tag="at", name=f"at_{hp}_{qc}")
            rr = atp.tile([65, 2, 512], F32, tag="rr", name=f"rr_{hp}_{qc}")
            nc.vector.tensor_copy(at, pv)            # releases pv banks
            # recip over all 65 partitions; only row 64 (rowsums) is consumed
            nc.vector.reciprocal_approx_fast(rr, at)
            if hp == 1:
                bc = pool.tile([128, 512], F32, tag="f0", name=f"bc_{hp}_{qc}")
                for h in range(2):
                    nc.tensor.matmul(
                        bc[64 * h:64 * h + 64, :], ones_sb[64:65, :],
                        rr[64:65, h, :], skip_group_check=True)
                for h in range(2):
                    nc.vector.tensor_mul(
                        attbf[h * 64:(h + 1) * 64, hp, qsl],
                        at[0:64, h, :], bc[64 * h:64 * h + 64, :])
                return
            nc.sync.dma_start(out=rs_dram[2 * hp:2 * hp + 2, qsl],
                              in_=rr[64:65, :, :])
            bq = [nc.sync, nc.gpsimd]
            for h in range(2):
                u4 = 2 * hp + h
                bc_ap = bass.AP(tensor=rs_dram, offset=u4 * TL + qc * 512,
                                ap=[[0, 64], [1, 512]])
                bq[h].dma_start(out=rsb_sb[:, u4, qsl], in_=bc_ap)
            for h in range(2):
                nc.vector.tensor_mul(
                    attbf[h * 64:(h + 1) * 64, hp, qsl],
                    at[0:64, h, :], rsb_sb[:, 2 * hp + h, qsl])

        def e_chunk(qc, mts, pool, three_q=False):
            """Output projection for one 512-token chunk, given mt tiles."""
            qsl = slice(qc * 512, (qc + 1) * 512)
            for mt in mts:
                pe = pool.tile([128, 512], F32, tag=f"f{mt % 2}", name=f"pe_{qc}_{mt}")
                for hp in range(2):
                    nc.tensor.matmul(
                        pe, wp[:, hp, mt * 128:(mt + 1) * 128], attbf[:, hp, qsl],
                        start=(hp == 0), stop=(hp == 1), skip_group_check=True)
                yt = ytp.tile([128, 512], BF16, tag="yt", name=f"yt_{qc}_{mt}")
                if three_q and mt % 2 == 0:
                    nc.scalar.copy(yt, pe)
                else:
                    nc.vector.tensor_copy(yt, pe)
                eng = ([nc.sync, nc.gpsimd, nc.scalar][mt % 3] if three_q
                       else [nc.sync, nc.gpsimd][mt % 2])
                eng.dma_start(
                    out=yT_d[mt * 128:(mt + 1) * 128, qsl], in_=yt)

        def phase_d(hp, pds, pdv, npool, fillers, in_loop=None,
                    boundary=None):
            """Attention for head pair hp with causal trimming.
            The PV pipeline runs a fixed 4-kt behind S/exp and crosses qc
            boundaries, so S/exp stream continuously while a qc's trailing
            PVs + normalize drain during the next qc's iterations.
            fillers: independent PE work consumed into exp-bound gaps.
            in_loop(qc, kt): extra emission hook (phase E chunks in D1).
            boundary(qc): emission hook at qc start."""
            nkt_total = sum(4 * (qc + 1) for qc in range(4))
            stride = max(1, nkt_total // max(1, len(fillers)))
            ktc = 0
            pvt = {}      # qc -> lazily allocated pv tile
            pend = []     # (qc, kt, pt, c0, n_kt) awaiting PV emission

            def emit_pv(eqc, kt, pt, c0, e_nkt):
                if eqc not in pvt:
                    pvt[eqc] = pdv.tile([128, 2, 512], F32, tag="pv",
                                        name=f"pv_{hp}_{eqc}")[0:65]
                pv = pvt[eqc]
                for h in range(2):
                    u = kt * 4 + hp * 2 + h
                    nc.tensor.matmul(
                        pv[:, h, c0:512], v_sb[:, u, :], pt[:, h, c0:512],
                        start=(kt == 0), stop=(kt == e_nkt - 1),
                        skip_group_check=True)
                if kt == e_nkt - 1:
                    normalize_qc(hp, eqc, pv, npool)
                    del pvt[eqc]

            for qc in range(4):
                if boundary is not None:
                    boundary(qc)
                qbase = qc * 512
                n_kt = 4 * (qc + 1)
                for kt in range(n_kt):
                    d = kt - 4 * qc
                    c0 = 128 * d if d >= 0 else 0
                    ksl = slice(kt * 128, (kt + 1) * 128)
                    sps = pds.tile([128, 2, 512], F32, tag="sps",
                                   name=f"sps_{hp}_{qc}_{kt}")
                    for h in range(2):
                        hsl = slice(h * 64, (h + 1) * 64)
                        nc.tensor.matmul(
                            sps[:, h, c0:512], k_sb[hsl, hp, ksl],
                            q_sb[hsl, hp, qbase + c0:qbase + 512],
                            skip_group_check=True)
                    pt = ptp.tile([128, 2, 512], BF16, tag="pt",
                                  name=f"pt_{hp}_{qc}_{kt}")
                    nc.scalar.activation(pt[:, :, c0:512], sps[:, :, c0:512],
                                         AF.Exp, bias=0.0, scale=0.125)
                    if d >= 0:  # triangular 128-col block of the diagonal tile
                        nc.vector.tensor_mul(
                            pt[:, :, c0:c0 + 128], pt[:, :, c0:c0 + 128],
                            masks.rearrange("p (a b) -> p a b", a=2))
                    pend.append((qc, kt, pt, c0, n_kt))
                    if len(pend) > 4:
                        emit_pv(*pend.pop(0))
                    ktc += 1
                    if fillers and ktc % stride == 0:
                        fillers.pop(0)()
                    if in_loop is not None:
                        in_loop(qc, kt)
            for args in pend:
                emit_pv(*args)

        with tc.tile_pool(name="pds", bufs=2, space="PSUM") as pds, \
             tc.tile_pool(name="pdv", bufs=1, space="PSUM") as pdv, \
             tc.tile_pool(name="fill", bufs=1, space="PSUM") as fill:
            warm_n = [0]

            def keep_alive(n, tag="f0"):
                """Junk matmuls: keep PE warm while real work waits on deps."""
                wt = fill.tile([128, 512], F32, tag=tag,
                               name=f"warm{warm_n[0]}")
                warm_n[0] += 1
                for _ in range(n):
                    nc.tensor.matmul(wt[:, 0:256], junk[:, 0:128],
                                     junk[:, 0:256],
                                     start=True, stop=True,
                                     skip_group_check=True)

            # PE warmup: junk matmuls so HAM reaches K=8/8 during the DMA phase
            keep_alive(38)

            # head-pair-0 QK for qc0 + v tiles for D0 qc0
            phase_b_block((2, 0), 0, fill)
            for tt in range(4):
                phase_c_tile(tt, fill)
            # All remaining B/C work goes into D0's per-kt filler stream,
            # deadline-ordered (consumed one per kt unit, 40 units total):
            #   qc0 u0-3:   B(0,1) B(2,1)        (q/k hp0 tc4=1, due u4/u8)
            #   qc1 u4-11:  C4-7, B(0,2) B(2,2)  (due u8+, u12, u20)
            #   qc2 u12-23: B(0,3), C8-11, B(2,3), B(3,0), B(1,0)
            #   qc3 u24-39: C12-15, B(3,1) B(1,1) B(3,2) B(1,2)
            # (m,tc4) = (1,3) and (3,3) are deferred to D1 qc boundaries.
            bf = lambda pairs: phase_b_fillers_for(pairs, fill)
            cf = lambda tts: [(lambda tt=tt: phase_c_tile(tt, fill))
                              for tt in tts]
            fillers = (bf([(0, 1), (2, 1)]) +
                       cf(range(4, 8)) + bf([(0, 2), (2, 2)]) +
                       bf([(0, 3)]) + cf(range(8, 12)) +
                       bf([(2, 3), (3, 0), (1, 0)]) +
                       cf(range(12, 16)) +
                       bf([(3, 1), (1, 1)]))

            phase_d(0, pds, pdv, fill, fillers)
            for f in list(fillers):   # flush any leftovers before D1
                f()
            fillers.clear()

            # D1: phase E chunks for qc-1 emitted into qc's loop; deferred B
            # chunks + junk keep PE warm across the pv-release stalls
            E_KTS = {1: (3, 4, 6, 7), 2: (3, 5, 7, 9), 3: (3, 6, 9, 12)}
            B_KTS = {(0, 1): (1, 2), (0, 3): (3, 2),
                     (1, 1): (1, 3), (1, 5): (3, 3)}

            def e_sched(qc, kt):
                if qc >= 1 and kt in E_KTS[qc]:
                    i = E_KTS[qc].index(kt)
                    e_chunk(qc - 1, (2 * i, 2 * i + 1), fill)
                bm = B_KTS.get((qc, kt))
                if bm is not None:
                    phase_b_block((bm[0],), bm[1], fill, fi0=bm[1] % 2)

            phase_d(1, pds, pdv, fill, [], in_loop=e_sched)
            keep_alive(20, tag="f1")
            e_chunk(3, range(0, 8), fill, three_q=True)
    nc.compile()
    return nc


def _rope_T():
    theta = 1.0 / (10000.0 ** (2.0 * np.arange(0, HD // 2, dtype=np.float32) / HD))
    seq = np.arange(1, T + 1, dtype=np.float32)
    ang = np.einsum('n,d->nd', seq, theta)
    ang = np.concatenate([ang, ang], axis=-1)
    f = (np.cos(ang) + np.sin(ang)).astype(np.float32)  # [T, 64]
    return np.concatenate([f.T, f.T], axis=0)           # [128, T]


def _host_inputs(x, W_attn, b_attn, W_proj, b_proj):
    bf = ml_dtypes.bfloat16
    ropeT = _rope_T().astype(bf)
    kp = np.arange(128)[:, None]
    j = np.arange(128)[None, :]
    tri = (kp <= j).astype(np.float32)
    masks = np.concatenate([tri, tri], axis=1).astype(bf)  # [128, 256]

    in_maps = []
    for c in range(8):
        b, jg = divmod(c, 4)
        hs = [4 * jg + i for i in range(4)]
        xT = np.ascontiguousarray(x[b].T).astype(bf)          # [1024, TL]
        q_rows = np.concatenate([W_attn[64 * h:64 * (h + 1)] for h in hs], 0)
        k_rows = np.concatenate([W_attn[C + 64 * h:C + 64 * (h + 1)] for h in hs], 0)
        WqkT = np.concatenate([q_rows, k_rows], 0).T          # [1024, 512]
        bqk = np.concatenate(
            [np.concatenate([b_attn[64 * h:64 * (h + 1)] for h in hs]),
             np.concatenate([b_attn[C + 64 * h:C + 64 * (h + 1)] for h in hs])])
        v_rows = np.concatenate([W_attn[2 * C + 64 * h:2 * C + 64 * (h + 1)] for h in hs], 0)
        WvT = v_rows.T                                        # [1024, 256]
        bv = np.concatenate([b_attn[2 * C + 64 * h:2 * C + 64 * (h + 1)] for h in hs])
        WpT = np.concatenate([W_proj[:, 64 * h:64 * (h + 1)] for h in hs], 1).T  # [256,1024]
        in_maps.append({
            "xT": np.ascontiguousarray(
                xT.reshape(NCT, 128, TL).transpose(1, 0, 2)),
            "wqkT": np.ascontiguousarray(
                WqkT.astype(bf).reshape(NCT, 128, 4, 128).transpose(1, 2, 0, 3)),
            "wvT": np.ascontiguousarray(
                WvT.astype(bf).reshape(NCT, 128, 256).transpose(1, 0, 2)),
            "bqk": np.ascontiguousarray(bqk.reshape(4, 128).T.astype(np.float32)),
            "bv": np.ascontiguousarray(
                np.broadcast_to(bv[None, :].astype(np.float32), (128, 256))),
            "rope": ropeT,
            "masks": masks,
            "wpT": np.ascontiguousarray(
                WpT.astype(bf).reshape(2, 128, 1024).transpose(1, 0, 2)),
        })
    return in_maps


def kernel(x, W_attn, b_attn, W_proj, b_proj):
    if "nc" not in _CACHE:
        _CACHE["nc"] = _build_nc()
    nc = _CACHE["nc"]
    in_maps = _host_inputs(x, W_attn, b_attn, W_proj, b_proj)
    res = run_bass_kernel_spmd(nc, in_maps, list(range(8)), trace=TRACE)
    _CACHE["last"] = res
    y = np.zeros((B, T, C), np.float32)
    for c in range(8):
        y[c // 4] += res.results[c]["yT"].astype(np.float32).T
    y += b_proj.astype(np.float32)
    return y


# revision 43
# speedup vs baseline: 1.0028x; 1.0028x over previous
"""Causal self-attention (B=2, T=2048, C=1024, NH=16, HD=64) on 8 TRN2 cores.

Sharding: core c -> batch b = c//4, head group j = c%4 (4 heads: 4j..4j+3).
Each core computes its batch's QKV projection for its 4 heads, rope, causal
flash-style attention in S^T layout (k on partitions, q on free dim), and a
partial output projection y_part^T = Wp_slice^T.T @ attbf. Host sums the 4
per-batch partials (bf16) and adds b_proj.

v2 schedule (vs v1):
  - xT DMA chunked per (tc4, ct) and ordered by consumption so phase B
    starts ~5us in instead of ~35us; 3 DGE queues (sync/gpsimd/vector).
  - PE warmup spinner at t=0 (junk matmuls) so HAM is at K=8/8 when real
    matmuls arrive.
  - Causal trimming: diagonal S/exp/PV restricted to cols >= 128d; mask
    multiply shrunk to the single 128-col triangular block (tri mask is
    d-independent).
  - Per-qc normalize fused from PV psum (rowsum row 64 -> recip -> DRAM
    bounce broadcast -> attbf = pv * recip), pipelined into the next qc.
  - Phase E (output proj) per-qc chunks interleaved into D1's exp gaps.
  - rope bf16, masks [128,256] bf16, yT output bf16.
"""
import numpy as np
import ml_dtypes
from contextlib import ExitStack

import concourse.bass as bass
import concourse.mybir as mybir
import concourse.tile as tile
from concourse import bacc
from concourse.bass_utils import run_bass_kernel_spmd

F32 = mybir.dt.float32
BF16 = mybir.dt.bfloat16
AF = mybir.ActivationFunctionType
ALU = mybir.AluOpType

B, T, C = 2, 2048, 1024
NH, HD = 16, 64
TL = 2048          # per-core token count (one batch)
NCT = C // 128     # 8 contraction tiles
NTC = TL // 512    # 4 t-chunks of 512
NTT = TL // 128    # 16 token tiles of 128

TRACE = False      # set by test harness for profiling runs
_CACHE = {}


def _build_nc():
    nc = bacc.Bacc("TRN2", target_bir_lowering=False, debug=False)
    xT_d = nc.dram_tensor("xT", [128, NCT, TL], BF16, kind="ExternalInput").ap()
    wqk_d = nc.dram_tensor("wqkT", [128, 4, NCT, 128], BF16, kind="ExternalInput").ap()
    wv_d = nc.dram_tensor("wvT", [128, NCT, 256], BF16, kind="ExternalInput").ap()
    bqk_d = nc.dram_tensor("bqk", [128, 4], F32, kind="ExternalInput").ap()
    bv_d = nc.dram_tensor("bv", [128, 256], F32, kind="ExternalInput").ap()
    rope_d = nc.dram_tensor("rope", [128, TL], BF16, kind="ExternalInput").ap()
    masks_d = nc.dram_tensor("masks", [128, 256], BF16, kind="ExternalInput").ap()
    wp_d = nc.dram_tensor("wpT", [128, 2, 1024], BF16, kind="ExternalInput").ap()
    yT_d = nc.dram_tensor("yT", [1024, TL], BF16, kind="ExternalOutput").ap()
    rs_dram = nc.dram_tensor("rs_scratch", [4, TL], F32)

    with tile.TileContext(nc) as tc, ExitStack() as ctx:
        sb = ctx.enter_context(tc.tile_pool(name="sb", bufs=1))
        ptp = ctx.enter_context(tc.tile_pool(name="ptp", bufs=8))
        ytp = ctx.enter_context(tc.tile_pool(name="ytp", bufs=8))
        atp = ctx.enter_context(tc.tile_pool(name="atp", bufs=2))

        xT = sb.tile([128, NCT, TL], BF16)
        wqk = sb.tile([128, 4, NCT, 128], BF16)
        wv = sb.tile([128, NCT, 256], BF16)
        bqk = sb.tile([128, 4], F32)
        bv = sb.tile([128, 256], F32)
        rope = sb.tile([128, TL], BF16)
        masks = sb.tile([128, 256], BF16)
        wp = sb.tile([128, 2, 1024], BF16)
        q_sb = sb.tile([128, 2, TL], BF16)
        k_sb = sb.tile([128, 2, TL], BF16)
        v_sb = sb.tile([128, 4 * NTT, 65], BF16)
        attbf = sb.tile([128, 2, TL], BF16)
        rsb_sb = sb.tile([64, 4, TL], F32)
        junk = sb.tile([128, 512], BF16)
        ones_sb = sb.tile([128, 64], F32)

        # ---- memsets (DVE/GpSimd, no deps) ----
        nc.vector.memset(junk, 0.0)
        nc.vector.memset(ones_sb, 1.0)
        nc.vector.memset(v_sb[:, :, 64:65], 1.0)  # ones col last: rowsum row 64

        # ---- input DMAs, ordered by consumption, across 3 DGE queues ----
        qs = [nc.sync, nc.gpsimd, nc.scalar]
        # B-block-0 deps first: wqk m-slices + xT tc4=0, then rope/wv
        nc.sync.dma_start(out=wqk[:, 2], in_=wqk_d[:, 2])
        nc.gpsimd.dma_start(out=wqk[:, 0], in_=wqk_d[:, 0])
        nc.scalar.dma_start(out=bqk, in_=bqk_d)
        for ct in range(NCT):                      # tc4=0 chunk of x
            qs[ct % 3].dma_start(out=xT[:, ct, 0:512], in_=xT_d[:, ct, 0:512])
        nc.scalar.dma_start(out=rope, in_=rope_d)
        nc.sync.dma_start(out=wv, in_=wv_d)
        nc.gpsimd.dma_start(out=bv, in_=bv_d)
        nc.scalar.dma_start(out=masks, in_=masks_d)
        nc.sync.dma_start(out=wqk[:, 3], in_=wqk_d[:, 3])
        nc.gpsimd.dma_start(out=wqk[:, 1], in_=wqk_d[:, 1])
        for tc4 in range(1, NTC):                  # remaining x chunks
            for ct in range(NCT):                  # sync/gpsimd only: keep
                sl = slice(tc4 * 512, (tc4 + 1) * 512)   # scalar free for exp
                qs[(tc4 * NCT + ct) % 2].dma_start(
                    out=xT[:, ct, sl], in_=xT_d[:, ct, sl])
        nc.gpsimd.dma_start(out=wp, in_=wp_d)

        def phase_b_block(ms, tc4, pool, fi0=0):
            """QK projection for one 512-col t-chunk of the given m-tiles.
            m: 0=q hp0, 1=q hp1, 2=k hp0, 3=k hp1."""
            sl = slice(tc4 * 512, (tc4 + 1) * 512)
            for fi, m in enumerate(ms, start=fi0):
                ps = pool.tile([128, 512], F32, tag=f"f{fi % 2}",
                               name=f"pbb_{m}_{tc4}")
                for ct in range(NCT):
                    nc.tensor.matmul(
                        ps, wqk[:, m, ct, :], xT[:, ct, sl],
                        start=(ct == 0), stop=(ct == NCT - 1),
                        skip_group_check=True)
                dest = q_sb if m < 2 else k_sb
                nc.vector.scalar_tensor_tensor(
                    out=dest[:, m % 2, sl], in0=ps, scalar=bqk[:, m:m + 1],
                    in1=rope[:, sl], op0=ALU.add, op1=ALU.mult)

        def phase_b_fillers_for(pairs, pool):
            """QK projection (m,tc4) pairs sliced into 4-MM filler chunks."""
            state = {}
            chunks = []
            for m, tc4 in pairs:
                    for half in range(2):
                        def emit(m=m, tc4=tc4, half=half):
                            sl = slice(tc4 * 512, (tc4 + 1) * 512)
                            if half == 0:
                                state[(m, tc4)] = pool.tile(
                                    [128, 512], F32, tag=f"f{(m + tc4) % 2}",
                                    name=f"pbf_{m}_{tc4}")
                            ps = state[(m, tc4)]
                            cts = (0, 1, 2, 3) if half == 0 else (4, 5, 6, 7)
                            for ct in cts:
                                nc.tensor.matmul(
                                    ps, wqk[:, m, ct, :],
                                    xT[:, ct, sl],
                                    start=(ct == 0), stop=(ct == NCT - 1),
                                    skip_group_check=True)
                            if half == 1:
                                dest = q_sb if m < 2 else k_sb
                                nc.vector.scalar_tensor_tensor(
                                    out=dest[:, m % 2, sl], in0=ps,
                                    scalar=bqk[:, m:m + 1], in1=rope[:, sl],
                                    op0=ALU.add, op1=ALU.mult)
                        chunks.append(emit)
            return chunks

        def phase_c_tile(tt, pool):
            """V projection for one token tile (8 MMs + fused bias add)."""
            ps = pool.tile([128, 512], F32, tag=f"f{tt % 2}", name=f"pc_{tt}")[:, 0:256]
            for ct in range(NCT):
                nc.tensor.matmul(
                    ps, xT[:, ct, tt * 128:(tt + 1) * 128], wv[:, ct, :],
                    start=(ct == 0), stop=(ct == NCT - 1),
                    skip_group_check=True)
            nc.vector.tensor_add(
                v_sb[:, 4 * tt:4 * tt + 4, 0:64],
                ps.rearrange("p (a b) -> p a b", a=4),
                bv.rearrange("p (a b) -> p a b", a=4))

        def normalize_qc(hp, qc, pv, pool):
            """One fast pv->sbuf copy (releases the psum tile), then recip +
            broadcast + scale off the psum path. pv rows: 0-63 = att dims,
            64 = rowsum (ones col last). Broadcast of the reciprocal row:
            DRAM bounce in D0 (PE is the wall there), PE ones-matmul in D1
            (PE has slack; kills ~4us of DMA latency per chain)."""
            qsl = slice(qc * 512, (qc + 1) * 512)
            at = atp.tile([65, 2, 512], F32, tag="at", name=f"at_{hp}_{qc}")
            rr = atp.tile([65, 2, 512], F32, tag="rr", name=f"rr_{hp}_{qc}")
            nc.vector.tensor_copy(at, pv)            # releases pv banks
            # recip over all 65 partitions; only row 64 (rowsums) is consumed
            nc.vector.reciprocal_approx_fast(rr, at)
            if hp == 1:
                bc = pool.tile([128, 512], F32, tag="f0", name=f"bc_{hp}_{qc}")
                for h in range(2):
                    nc.tensor.matmul(
                        bc[64 * h:64 * h + 64, :], ones_sb[64:65, :],
                        rr[64:65, h, :], skip_group_check=True)
                for h in range(2):
                    nc.vector.tensor_mul(
                        attbf[h * 64:(h + 1) * 64, hp, qsl],
                        at[0:64, h, :], bc[64 * h:64 * h + 64, :])
                return
            nc.sync.dma_start(out=rs_dram[2 * hp:2 * hp + 2, qsl],
                              in_=rr[64:65, :, :])
            bq = [nc.sync, nc.gpsimd]
            for h in range(2):
                u4 = 2 * hp + h
                bc_ap = bass.AP(tensor=rs_dram, offset=u4 * TL + qc * 512,
                                ap=[[0, 64], [1, 512]])
                bq[h].dma_start(out=rsb_sb[:, u4, qsl], in_=bc_ap)
            for h in range(2):
                nc.vector.tensor_mul(
                    attbf[h * 64:(h + 1) * 64, hp, qsl],
                    at[0:64, h, :], rsb_sb[:, 2 * hp + h, qsl])

        def e_chunk(qc, mts, pool, three_q=False):
            """Output projection for one 512-token chunk, given mt tiles."""
            qsl = slice(qc * 512, (qc + 1) * 512)
            for mt in mts:
                pe = pool.tile([128, 512], F32, tag=f"f{mt % 2}", name=f"pe_{qc}_{mt}")
                for hp in range(2):
                    nc.tensor.matmul(
                        pe, wp[:, hp, mt * 128:(mt + 1) * 128], attbf[:, hp, qsl],
                        start=(hp == 0), stop=(hp == 1), skip_group_check=True)
                yt = ytp.tile([128, 512], BF16, tag="yt", name=f"yt_{qc}_{mt}")
                if three_q and mt % 2 == 0:
                    nc.scalar.copy(yt, pe)
                else:
                    nc.vector.tensor_copy(yt, pe)
                eng = ([nc.sync, nc.scalar][mt % 2] if three_q
                       else [nc.sync, nc.gpsimd][mt % 2])
                eng.dma_start(
                    out=yT_d[mt * 128:(mt + 1) * 128, qsl], in_=yt)

        def phase_d(hp, pds, pdv, npool, fillers, in_loop=None,
                    boundary=None):
            """Attention for head pair hp with causal trimming.
            The PV pipeline runs a fixed 4-kt behind S/exp and crosses qc
            boundaries, so S/exp stream continuously while a qc's trailing
            PVs + normalize drain during the next qc's iterations.
            fillers: independent PE work consumed into exp-bound gaps.
            in_loop(qc, kt): extra emission hook (phase E chunks in D1).
            boundary(qc): emission hook at qc start."""
            nkt_total = sum(4 * (qc + 1) for qc in range(4))
            stride = max(1, nkt_total // max(1, len(fillers)))
            ktc = 0
            pvt = {}      # qc -> lazily allocated pv tile
            pend = []     # (qc, kt, pt, c0, n_kt) awaiting PV emission

            def emit_pv(eqc, kt, pt, c0, e_nkt):
                if eqc not in pvt:
                    pvt[eqc] = pdv.tile([128, 2, 512], F32, tag="pv",
                                        name=f"pv_{hp}_{eqc}")[0:65]
                pv = pvt[eqc]
                for h in range(2):
                    u = kt * 4 + hp * 2 + h
                    nc.tensor.matmul(
                        pv[:, h, c0:512], v_sb[:, u, :], pt[:, h, c0:512],
                        start=(kt == 0), stop=(kt == e_nkt - 1),
                        skip_group_check=True)
                if kt == e_nkt - 1:
                    normalize_qc(hp, eqc, pv, npool)
                    del pvt[eqc]

            for qc in range(4):
                if boundary is not None:
                    boundary(qc)
                qbase = qc * 512
                n_kt = 4 * (qc + 1)
                for kt in range(n_kt):
                    d = kt - 4 * qc
                    c0 = 128 * d if d >= 0 else 0
                    ksl = slice(kt * 128, (kt + 1) * 128)
                    sps = pds.tile([128, 2, 512], F32, tag="sps",
                                   name=f"sps_{hp}_{qc}_{kt}")
                    for h in range(2):
                        hsl = slice(h * 64, (h + 1) * 64)
                        nc.tensor.matmul(
                            sps[:, h, c0:512], k_sb[hsl, hp, ksl],
                            q_sb[hsl, hp, qbase + c0:qbase + 512],
                            skip_group_check=True)
                    pt = ptp.tile([128, 2, 512], BF16, tag="pt",
                                  name=f"pt_{hp}_{qc}_{kt}")
                    nc.scalar.activation(pt[:, :, c0:512], sps[:, :, c0:512],
                                         AF.Exp, bias=0.0, scale=0.125)
                    if d >= 0:  # triangular 128-col block of the diagonal tile
                        nc.vector.tensor_mul(
                            pt[:, :, c0:c0 + 128], pt[:, :, c0:c0 + 128],
                            masks.rearrange("p (a b) -> p a b", a=2))
                    pend.append((qc, kt, pt, c0, n_kt))
                    if len(pend) > 4:
                        emit_pv(*pend.pop(0))
                    ktc += 1
                    if fillers and ktc % stride == 0:
                        fillers.pop(0)()
                    if in_loop is not None:
                        in_loop(qc, kt)
            for args in pend:
                emit_pv(*args)

        with tc.tile_pool(name="pds", bufs=2, space="PSUM") as pds, \
             tc.tile_pool(name="pdv", bufs=1, space="PSUM") as pdv, \
             tc.tile_pool(name="fill", bufs=1, space="PSUM") as fill:
            warm_n = [0]

            def keep_alive(n, tag="f0"):
                """Junk matmuls: keep PE warm while real work waits on deps."""
                wt = fill.tile([128, 512], F32, tag=tag,
                               name=f"warm{warm_n[0]}")
                warm_n[0] += 1
                for _ in range(n):
                    nc.tensor.matmul(wt[:, 0:256], junk[:, 0:128],
                                     junk[:, 0:256],
                                     start=True, stop=True,
                                     skip_group_check=True)

            # PE warmup: junk matmuls so HAM reaches K=8/8 during the DMA phase
            keep_alive(38)

            # head-pair-0 QK for qc0 + v tiles for D0 qc0
            phase_b_block((2, 0), 0, fill)
            for tt in range(4):
                phase_c_tile(tt, fill)
            # All remaining B/C work goes into D0's per-kt filler stream,
            # deadline-ordered (consumed one per kt unit, 40 units total):
            #   qc0 u0-3:   B(0,1) B(2,1)        (q/k hp0 tc4=1, due u4/u8)
            #   qc1 u4-11:  C4-7, B(0,2) B(2,2)  (due u8+, u12, u20)
            #   qc2 u12-23: B(0,3), C8-11, B(2,3), B(3,0), B(1,0)
            #   qc3 u24-39: C12-15, B(3,1) B(1,1) B(3,2) B(1,2)
            # (m,tc4) = (1,3) and (3,3) are deferred to D1 qc boundaries.
            bf = lambda pairs: phase_b_fillers_for(pairs, fill)
            cf = lambda tts: [(lambda tt=tt: phase_c_tile(tt, fill))
                              for tt in tts]
            fillers = (bf([(0, 1), (2, 1)]) +
                       cf(range(4, 8)) + bf([(0, 2), (2, 2)]) +
                       bf([(0, 3)]) + cf(range(8, 12)) +
                       bf([(2, 3), (3, 0), (1, 0)]) +
                       cf(range(12, 16)) +
                       bf([(3, 1), (1, 1)]))

            phase_d(0, pds, pdv, fill, fillers)
            for f in list(fillers):   # flush any leftovers before D1
                f()
            fillers.clear()

            # D1: phase E chunks for qc-1 emitted into qc's loop; deferred B
            # chunks + junk keep PE warm across the pv-release stalls
            E_KTS = {1: (3, 4, 6, 7), 2: (3, 5, 7, 9), 3: (3, 6, 9, 12)}
            B_KTS = {(0, 1): (1, 2), (0, 3): (3, 2),
                     (1, 1): (1, 3), (1, 5): (3, 3)}

            def e_sched(qc, kt):
                if qc >= 1 and kt in E_KTS[qc]:
                    i = E_KTS[qc].index(kt)
                    e_chunk(qc - 1, (2 * i, 2 * i + 1), fill)
                bm = B_KTS.get((qc, kt))
                if bm is not None:
                    phase_b_block((bm[0],), bm[1], fill, fi0=bm[1] % 2)

            phase_d(1, pds, pdv, fill, [], in_loop=e_sched)
            keep_alive(26, tag="f1")
            e_chunk(3, range(0, 8), fill, three_q=True)
    nc.compile()
    return nc


def _rope_T():
    theta = 1.0 / (10000.0 ** (2.0 * np.arange(0, HD // 2, dtype=np.float32) / HD))
    seq = np.arange(1, T + 1, dtype=np.float32)
    ang = np.einsum('n,d->nd', seq, theta)
    ang = np.concatenate([ang, ang], axis=-1)
    f = (np.cos(ang) + np.sin(ang)).astype(np.float32)  # [T, 64]
    return np.concatenate([f.T, f.T], axis=0)           # [128, T]


def _host_inputs(x, W_attn, b_attn, W_proj, b_proj):
    bf = ml_dtypes.bfloat16
    ropeT = _rope_T().astype(bf)
    kp = np.arange(128)[:, None]
    j = np.arange(128)[None, :]
    tri = (kp <= j).astype(np.float32)
    masks = np.concatenate([tri, tri], axis=1).astype(bf)  # [128, 256]

    in_maps = []
    for c in range(8):
        b, jg = divmod(c, 4)
        hs = [4 * jg + i for i in range(4)]
        xT = np.ascontiguousarray(x[b].T).astype(bf)          # [1024, TL]
        q_rows = np.concatenate([W_attn[64 * h:64 * (h + 1)] for h in hs], 0)
        k_rows = np.concatenate([W_attn[C + 64 * h:C + 64 * (h + 1)] for h in hs], 0)
        WqkT = np.concatenate([q_rows, k_rows], 0).T          # [1024, 512]
        bqk = np.concatenate(
            [np.concatenate([b_attn[64 * h:64 * (h + 1)] for h in hs]),
             np.concatenate([b_attn[C + 64 * h:C + 64 * (h + 1)] for h in hs])])
        v_rows = np.concatenate([W_attn[2 * C + 64 * h:2 * C + 64 * (h + 1)] for h in hs], 0)
        WvT = v_rows.T                                        # [1024, 256]
        bv = np.concatenate([b_attn[2 * C + 64 * h:2 * C + 64 * (h + 1)] for h in hs])
        WpT = np.concatenate([W_proj[:, 64 * h:64 * (h + 1)] for h in hs], 1).T  # [256,1024]
        in_maps.append({
            "xT": np.ascontiguousarray(
                xT.reshape(NCT, 128, TL).transpose(1, 0, 2)),
            "wqkT": np.ascontiguousarray(
                WqkT.astype(bf).reshape(NCT, 128, 4, 128).transpose(1, 2, 0, 3)),
            "wvT": np.ascontiguousarray(
                WvT.astype(bf).reshape(NCT, 128, 256).transpose(1, 0, 2)),
            "bqk": np.ascontiguousarray(bqk.reshape(4, 128).T.astype(np.float32)),
            "bv": np.ascontiguousarray(
                np.broadcast_to(bv[None, :].astype(np.float32), (128, 256))),
            "rope": ropeT,
            "masks": masks,
            "wpT": np.ascontiguousarray(
                WpT.astype(bf).reshape(2, 128, 1024).transpose(1, 0, 2)),
        })
    return in_maps


def kernel(x, W_attn, b_attn, W_proj, b_proj):
    if "nc" not in _CACHE:
        _CACHE["nc"] = _build_nc()
    nc = _CACHE["nc"]
    in_maps = _host_inputs(x, W_attn, b_attn, W_proj, b_proj)
    res = run_bass_kernel_spmd(nc, in_maps, list(range(8)), trace=TRACE)
    _CACHE["last"] = res
    y = np.zeros((B, T, C), np.float32)
    for c in range(8):
        y[c // 4] += res.results[c]["yT"].astype(np.float32).T
    y += b_proj.astype(np.float32)
    return y
